# revision 1
# baseline (speedup 1.0000x reference)
"""CCNF RK4 sampling kernel for 8 Trainium2 NeuronCores.

Data-parallel: batch 2048 -> 256 per core, weights replicated.
On-core layout: features on partitions, batch on the free dim (N=256).
Matmuls run in float32r (fp32 data, fast PE mode).
"""

import os

import numpy as np

N_CORES = 8


def _build_program(theta0, context, W1, b1, W2, b2, W3, b3, n_steps):
    import concourse.bass as bass
    import concourse.mybir as mybir
    import concourse.tile as tile
    from concourse import bacc
    from concourse.bass_utils import run_bass_kernel_spmd

    f32 = mybir.dt.float32
    f32r = mybir.dt.float32r
    ALU = mybir.AluOpType
    SIGMOID = mybir.ActivationFunctionType.Sigmoid

    B, D = theta0.shape          # 2048, 32
    C = context.shape[1]         # 128
    IN, H2 = W1.shape            # 161, 1024
    H = W2.shape[0]              # 512
    assert H2 == 2 * H and W2.shape[1] == 2 * H and W3.shape == (H, D)
    assert IN == D + 1 + C
    assert B % N_CORES == 0
    Bs = B // N_CORES            # 256 per core
    steps = int(n_steps)
    dt = 1.0 / steps

    KC = H // 128                # 4 k-chunks for layer 2/3
    MJ = H // 128                # 4 output column-chunks per GLU half
    # layer-1 K split: rows [0:33) = theta(32)+t(1) (dynamic); rows [33:161) = ctx (static)
    K1A = D + 1                  # 33 (theta + t)
    K1B = IN - K1A               # 128 (ctx)

    # ---- host-side layout prep (shared across cores) ----
    W1 = np.ascontiguousarray(W1, np.float32)
    w1c1_h = np.ascontiguousarray(W1[:K1A])                    # [33, 1024]  theta+t rows
    w1c2_h = np.ascontiguousarray(W1[K1A:])                    # [128, 1024] ctx rows
    b3_is_zero = not np.any(np.asarray(b3, np.float32))
    # [512,1024] -> [128, 4*1024]: chunk kc at cols kc*1024
    w2_h = np.ascontiguousarray(
        np.asarray(W2, np.float32).reshape(KC, 128, 2 * H).transpose(1, 0, 2).reshape(128, KC * 2 * H)
    )
    # [512,32] -> [128, 4*32]
    w3_h = np.ascontiguousarray(
        np.asarray(W3, np.float32).reshape(KC, 128, D).transpose(1, 0, 2).reshape(128, KC * D)
    )
    b1 = np.asarray(b1, np.float32)
    b2 = np.asarray(b2, np.float32)
    bias_h = np.ascontiguousarray(np.concatenate([
        b1[:H].reshape(MJ, 128).T, b1[H:].reshape(MJ, 128).T,
        b2[:H].reshape(MJ, 128).T, b2[H:].reshape(MJ, 128).T,
    ], axis=1))                                                # [128, 16]
    onesb3_h = np.ascontiguousarray(np.concatenate([
        np.ones((1, Bs), np.float32),
        np.asarray(b3, np.float32).reshape(1, D),
    ], axis=1))                                                # [1, Bs+32]

    # ---- build the bass program (same program on all 8 cores) ----
    nc = bacc.Bacc("TRN2", target_bir_lowering=False)

    d_x1i = nc.dram_tensor("x1i", [K1A, Bs], f32r, kind="ExternalInput")  # theta rows + t row
    d_x2i = nc.dram_tensor("x2i", [K1B, Bs], f32r, kind="ExternalInput")  # ctx (static)
    d_th0 = nc.dram_tensor("th0", [D, Bs], f32r, kind="ExternalInput")
    d_w1c1 = nc.dram_tensor("w1c1", [K1A, 2 * H], f32r, kind="ExternalInput")
    d_w1c2 = nc.dram_tensor("w1c2", [K1B, 2 * H], f32r, kind="ExternalInput")
    d_w2 = nc.dram_tensor("w2", [128, KC * 2 * H], f32r, kind="ExternalInput")
    d_w3 = nc.dram_tensor("w3", [128, KC * D], f32r, kind="ExternalInput")
    d_bias = nc.dram_tensor("biases", [128, 4 * MJ], f32, kind="ExternalInput")
    d_ob3 = nc.dram_tensor("onesb3", [1, Bs + D], f32r, kind="ExternalInput")
    d_out = nc.dram_tensor("out", [D, Bs], f32, kind="ExternalOutput")

    # RK4 coefficients: arg scale (for next eval's input), acc scale
    c_arg = [0.5 * dt, 0.5 * dt, dt]
    a_acc = [dt / 6.0, dt / 3.0, dt / 3.0, dt / 6.0]

    with tile.TileContext(nc) as tc:
        PS3_SHARE = int(os.environ.get("KERNEL_PS3SHARE", "0"))
        PSMM_BUFS = 8 if PS3_SHARE else 7
        with (
            tc.tile_pool(name="const", bufs=1) as cpool,
            tc.tile_pool(name="psmm", bufs=PSMM_BUFS, space="PSUM") as ps_pool,
            tc.tile_pool(name="ps3", bufs=1, space="PSUM") as ps3_pool,
            tc.tile_pool(name="sig", bufs=int(os.environ.get("KERNEL_SIGB", "10"))) as sig_pool,
            tc.tile_pool(name="hact", bufs=int(os.environ.get("KERNEL_HB", "20"))) as h_pool,
            tc.tile_pool(name="accp", bufs=int(os.environ.get("KERNEL_AB", "6"))) as acc_pool,
        ):
            tw1c1 = cpool.tile([K1A, 2 * H], f32r)
            tw1c2 = cpool.tile([K1B, 2 * H], f32r)
            tw2 = cpool.tile([128, KC * 2 * H], f32r)
            tw3 = cpool.tile([128, KC * D], f32r)
            tbias = cpool.tile([128, 4 * MJ], f32)
            tb1a = tbias[:, 0 * MJ : 1 * MJ]
            tb1b = tbias[:, 1 * MJ : 2 * MJ]
            tb2a = tbias[:, 2 * MJ : 3 * MJ]
            tb2b = tbias[:, 3 * MJ : 4 * MJ]
            tob3 = cpool.tile([1, Bs + D], f32r)
            tones = tob3[:, 0:Bs]
            tb3 = tob3[:, Bs : Bs + D]
            tx1 = cpool.tile([K1A, Bs], f32r)   # rows: [theta(32) | t(1)]  (dynamic)
            tx2 = cpool.tile([K1B, Bs], f32r)   # ctx (static)
            tth0 = cpool.tile([D, Bs], f32r)    # initial theta

            # layer-1-critical tensors first so eval 0 can start while
            # w2/w3 still stream
            nc.sync.dma_start(tx2[:], d_x2i[:])
            nc.sync.dma_start(tw1c2[:], d_w1c2[:])
            nc.sync.dma_start(tx1[:], d_x1i[:])
            nc.sync.dma_start(tw1c1[:], d_w1c1[:])
            nc.sync.dma_start(tbias[:], d_bias[:])
            nc.sync.dma_start(tth0[:], d_th0[:])
            nc.sync.dma_start(tw2[:], d_w2[:])
            nc.sync.dma_start(tw3[:], d_w3[:])
            nc.sync.dma_start(tob3[:], d_ob3[:])

            def mm(out_ap, lhsT_ap, rhs_ap, start, stop):
                nc.tensor.matmul(out_ap, lhsT_ap, rhs_ap, start=start, stop=stop)

            th_cur = tth0       # theta at start of current step
            t_written = 0.0     # t-row was preloaded with 0

            def issue_l1ctx():
                # static context contribution for the NEXT eval's layer 1 --
                # issued early so PE has work during the RK4 latency chain.
                # One accumulation group per PSUM bank: only the first MM may
                # carry start=True (a second start would zero the whole bank).
                tiles = []
                for j in range(MJ):
                    ps = ps_pool.tile([128, 2 * Bs], f32, tag="psmm")
                    for half, mj in ((1, j + MJ), (0, j)):
                        dst = ps[:, half * Bs : (half + 1) * Bs]
                        msl = slice(mj * 128, (mj + 1) * 128)
                        mm(dst, tw1c2[:, msl], tx2[:],
                           start=(half == 1), stop=False)
                    tiles.append(ps)
                return tiles

            SCHED = int(os.environ.get("KERNEL_SCHED", "3"))
            SPLITP = int(os.environ.get("KERNEL_SPLITPOOLS", "1"))
            SIG1T, SIG2T = ("sig1", "sig2") if SPLITP else ("sig", "sig")
            H1T, H2T = ("h1t", "h2t") if SPLITP else ("hact", "hact")
            if SCHED >= 2 or SCHED == 3:
                ps1 = issue_l1ctx()

            TOFF = (0.0, 0.5, 0.5, 1.0)
            for s in range(steps):
                for e in range(4):
                    TMEMSET_MODE = int(os.environ.get("KERNEL_TMEMSET", "0")) if SCHED == 3 else 0
                    TMEMSET_TAIL = TMEMSET_MODE == 1
                    TMEMSET_POST = TMEMSET_MODE == 2
                    TMEMSET_POOL_TAIL = TMEMSET_MODE == 3
                    t_val = (s + TOFF[e]) * dt
                    if TMEMSET_MODE == 0 and t_val != t_written:
                        nc.gpsimd.memset(tx1[D : D + 1, :].bitcast(f32), float(t_val))
                        t_written = t_val

                    last_eval = (s == steps - 1) and (e == 3)

                    if SCHED == 3:
                        # v1.5 structure, ctx MMs pre-issued (ps1 tiles);
                        # group stop goes on the LAST MM of the bank (a-half)
                        h1 = []
                        for j in range(MJ):
                            ps = ps1[j]
                            for half, mj in ((1, j + MJ), (0, j)):
                                dst = ps[:, half * Bs : (half + 1) * Bs]
                                msl = slice(mj * 128, (mj + 1) * 128)
                                mm(dst, tw1c1[:, msl], tx1[:], start=False,
                                   stop=(half == 0))
                            sg = sig_pool.tile([128, Bs], f32, tag=SIG1T)
                            nc.scalar.activation(
                                sg[:], ps[:, Bs : 2 * Bs], SIGMOID,
                                bias=tb1b[:, j : j + 1]
                            )
                            ht = h_pool.tile([128, Bs], f32r, tag=H1T)
                            nc.vector.scalar_tensor_tensor(
                                ht[:], ps[:, 0:Bs], tb1a[:, j : j + 1], sg[:],
                                ALU.add, ALU.mult,
                            )
                            h1.append(ht)
                        h2 = []
                        for j in range(MJ):
                            ps = ps_pool.tile([128, 2 * Bs], f32, tag="psmm")
                            # b-half group first so the sigmoid overlaps the
                            # a-half matmuls (keeps ACT off the eval tail)
                            dstb = ps[:, Bs : 2 * Bs]
                            for kc in range(KC):
                                csl = slice(kc * 2 * H + (j + MJ) * 128,
                                            kc * 2 * H + (j + MJ + 1) * 128)
                                mm(dstb, tw2[:, csl], h1[kc][:],
                                   start=(kc == 0), stop=(kc == KC - 1))
                            sg = sig_pool.tile([128, Bs], f32, tag=SIG2T)
                            nc.scalar.activation(
                                sg[:], dstb, SIGMOID, bias=tb2b[:, j : j + 1]
                            )
                            dsta = ps[:, 0:Bs]
                            for kc in range(KC):
                                csl = slice(kc * 2 * H + j * 128,
                                            kc * 2 * H + (j + 1) * 128)
                                mm(dsta, tw2[:, csl], h1[kc][:],
                                   start=(kc == 0), stop=(kc == KC - 1))
                            ht = h_pool.tile([128, Bs], f32r, tag=H2T)
                            nc.vector.scalar_tensor_tensor(
                                ht[:], dsta, tb2a[:, j : j + 1], sg[:],
                                ALU.add, ALU.mult,
                            )
                            h2.append(ht)
                        if not last_eval:
                            ps1_next = issue_l1ctx()
                    elif SCHED == 0:
                        # v1.5: per-j, both halves, GLU immediately
                        h1 = []
                        for j in range(MJ):
                            ps = ps_pool.tile([128, 2 * Bs], f32, tag="psmm")
                            for half, mj in ((0, j), (1, j + MJ)):
                                dst = ps[:, half * Bs : (half + 1) * Bs]
                                msl = slice(mj * 128, (mj + 1) * 128)
                                mm(dst, tw1c2[:, msl], tx2[:], start=True, stop=False)
                                mm(dst, tw1c1[:, msl], tx1[:], start=False, stop=True)
                            sg = sig_pool.tile([128, Bs], f32, tag="sig")
                            nc.scalar.activation(
                                sg[:], ps[:, Bs : 2 * Bs], SIGMOID,
                                bias=tb1b[:, j : j + 1]
                            )
                            ht = h_pool.tile([128, Bs], f32r, tag="hact")
                            nc.vector.scalar_tensor_tensor(
                                ht[:], ps[:, 0:Bs], tb1a[:, j : j + 1], sg[:],
                                ALU.add, ALU.mult,
                            )
                            h1.append(ht)
                        h2 = []
                        for j in range(MJ):
                            ps = ps_pool.tile([128, 2 * Bs], f32, tag="psmm")
                            for half, mj in ((0, j), (1, j + MJ)):
                                dst = ps[:, half * Bs : (half + 1) * Bs]
                                for kc in range(KC):
                                    csl = slice(kc * 2 * H + mj * 128,
                                                kc * 2 * H + (mj + 1) * 128)
                                    mm(dst, tw2[:, csl], h1[kc][:],
                                       start=(kc == 0), stop=(kc == KC - 1))
                            sg = sig_pool.tile([128, Bs], f32, tag="sig")
                            nc.scalar.activation(
                                sg[:], ps[:, Bs : 2 * Bs], SIGMOID,
                                bias=tb2b[:, j : j + 1]
                            )
                            ht = h_pool.tile([128, Bs], f32r, tag="hact")
                            nc.vector.scalar_tensor_tensor(
                                ht[:], ps[:, 0:Bs], tb2a[:, j : j + 1], sg[:],
                                ALU.add, ALU.mult,
                            )
                            h2.append(ht)
                    else:
                        if SCHED < 2:
                            ps1 = issue_l1ctx()
                        sg1 = []
                        for j in range(MJ):
                            dst = ps1[j][:, Bs : 2 * Bs]
                            msl = slice((j + MJ) * 128, (j + MJ + 1) * 128)
                            mm(dst, tw1c1[:, msl], tx1[:], start=False, stop=True)
                            sg = sig_pool.tile([128, Bs], f32, tag="sig")
                            nc.scalar.activation(
                                sg[:], dst, SIGMOID, bias=tb1b[:, j : j + 1]
                            )
                            sg1.append(sg)
                        h1 = []
                        for j in range(MJ):
                            dst = ps1[j][:, 0:Bs]
                            msl = slice(j * 128, (j + 1) * 128)
                            mm(dst, tw1c1[:, msl], tx1[:], start=False, stop=True)
                            ht = h_pool.tile([128, Bs], f32r, tag="hact")
                            nc.vector.scalar_tensor_tensor(
                                ht[:], dst, tb1a[:, j : j + 1], sg1[j][:],
                                ALU.add, ALU.mult,
                            )
                            h1.append(ht)
                        ps2 = []
                        for j in range(MJ):
                            ps2j = ps_pool.tile([128, 2 * Bs], f32, tag="psmm")
                            ps2.append(ps2j)
                        for kc in range(KC - 1):
                            for j in range(MJ):
                                csl = slice(kc * 2 * H + (j + MJ) * 128,
                                            kc * 2 * H + (j + MJ + 1) * 128)
                                mm(ps2[j][:, Bs : 2 * Bs], tw2[:, csl], h1[kc][:],
                                   start=(kc == 0), stop=False)
                            for j in range(MJ):
                                csl = slice(kc * 2 * H + j * 128,
                                            kc * 2 * H + (j + 1) * 128)
                                mm(ps2[j][:, 0:Bs], tw2[:, csl], h1[kc][:],
                                   start=(kc == 0), stop=False)
                        kc = KC - 1
                        sg2 = []
                        for j in range(MJ):
                            csl = slice(kc * 2 * H + (j + MJ) * 128,
                                        kc * 2 * H + (j + MJ + 1) * 128)
                            dst = ps2[j][:, Bs : 2 * Bs]
                            mm(dst, tw2[:, csl], h1[kc][:], start=False, stop=True)
                            sg = sig_pool.tile([128, Bs], f32, tag="sig")
                            nc.scalar.activation(
                                sg[:], dst, SIGMOID, bias=tb2b[:, j : j + 1]
                            )
                            sg2.append(sg)
                        h2 = []
                        for j in range(MJ):
                            csl = slice(kc * 2 * H + j * 128,
                                        kc * 2 * H + (j + 1) * 128)
                            dst = ps2[j][:, 0:Bs]
                            mm(dst, tw2[:, csl], h1[kc][:], start=False, stop=True)
                            ht = h_pool.tile([128, Bs], f32r, tag="hact")
                            nc.vector.scalar_tensor_tensor(
                                ht[:], dst, tb2a[:, j : j + 1], sg2[j][:],
                                ALU.add, ALU.mult,
                            )
                            h2.append(ht)
                        if SCHED >= 2 and not last_eval:
                            ps1_next = issue_l1ctx()

                    # ---- layer 3: k = h2 @ W3 (+ b3) in PSUM ----
                    if PS3_SHARE:
                        ps3full = ps_pool.tile([128, 2 * Bs], f32, tag="psmm")
                        ps3 = ps3full[0:D, 0:Bs]
                    else:
                        ps3 = ps3_pool.tile([D, Bs], f32, tag="ps3")
                    for kc in range(KC):
                        mm(ps3[:], tw3[:, kc * D : (kc + 1) * D], h2[kc][:],
                           start=(kc == 0), stop=(kc == KC - 1 and b3_is_zero))
                    if not b3_is_zero:
                        mm(ps3[:], tb3[:], tones[:], start=False, stop=True)

                    # ---- RK4 bookkeeping ----
                    if TMEMSET_POOL_TAIL and not last_eval:
                        nxt_s, nxt_e = (s, e + 1) if e < 3 else (s + 1, 0)
                        nxt_t = (nxt_s + TOFF[nxt_e]) * dt
                        if nxt_t != t_written:
                            nc.gpsimd.memset(
                                tx1[D : D + 1, :].bitcast(f32), float(nxt_t)
                            )
                            t_written = nxt_t
                    if TMEMSET_TAIL and not last_eval:
                        # write the NEXT eval's t-row on DVE (same engine as
                        # the arg STT -> no extra cross-engine hop on the
                        # arg -> layer-1 chain)
                        nxt_s, nxt_e = (s, e + 1) if e < 3 else (s + 1, 0)
                        nxt_t = (nxt_s + TOFF[nxt_e]) * dt
                        if nxt_t != t_written:
                            nc.vector.memset(
                                tx1[D : D + 1, :].bitcast(f32), float(nxt_t)
                            )
                            t_written = nxt_t
                    if e < 3:
                        nc.vector.scalar_tensor_tensor(
                            tx1[0:D, :], ps3[:], float(c_arg[e]), th_cur[:],
                            ALU.mult, ALU.add,
                        )
                    if TMEMSET_POST and not last_eval:
                        nxt_s, nxt_e = (s, e + 1) if e < 3 else (s + 1, 0)
                        nxt_t = (nxt_s + TOFF[nxt_e]) * dt
                        if nxt_t != t_written:
                            nc.vector.memset(
                                tx1[D : D + 1, :].bitcast(f32), float(nxt_t)
                            )
                            t_written = nxt_t
                    base = th_cur if e == 0 else acc_prev
                    if e == 3 and s != steps - 1:
                        # theta_{s+1} goes straight into the matmul input tile
                        # (keeps the Pool copy off the critical chain)...
                        nc.vector.scalar_tensor_tensor(
                            tx1[0:D, :], ps3[:], float(a_acc[e]), base[:],
                            ALU.mult, ALU.add,
                        )
                    acc_new = acc_pool.tile([D, Bs], f32, tag="accp")
                    # ...and also into its own tile (used as th_cur next step)
                    nc.vector.scalar_tensor_tensor(
                        acc_new[:], ps3[:], float(a_acc[e]), base[:],
                        ALU.mult, ALU.add,
                    )
                    acc_prev = acc_new
                    if SCHED >= 2 and not last_eval:
                        ps1 = ps1_next

                th_cur = acc_prev  # theta_{s+1}

            nc.sync.dma_start(d_out[:], th_cur[:])

    # ---- per-core input maps ----
    in_maps = []
    for c in range(N_CORES):
        sl = slice(c * Bs, (c + 1) * Bs)
        th_T = np.ascontiguousarray(np.asarray(theta0[sl], np.float32).T)
        ctx_T = np.ascontiguousarray(np.asarray(context[sl], np.float32).T)
        x1i = np.concatenate([th_T, np.zeros((1, Bs), np.float32)], axis=0)
        in_maps.append(
            {
                "x1i": np.ascontiguousarray(x1i),
                "x2i": ctx_T,
                "th0": th_T,
                "w1c1": w1c1_h,
                "w1c2": w1c2_h,
                "w2": w2_h,
                "w3": w3_h,
                "biases": bias_h,
                "onesb3": onesb3_h,
            }
        )

    return nc, in_maps


def _build_and_run(theta0, context, W1, b1, W2, b2, W3, b3, n_steps):
    from concourse.bass_utils import run_bass_kernel_spmd

    nc, in_maps = _build_program(theta0, context, W1, b1, W2, b2, W3, b3, n_steps)
    nc.finalize()  # Bacc: split multi-sem waits + allocate registers
    res = run_bass_kernel_spmd(
        nc,
        in_maps,
        core_ids=list(range(N_CORES)),
        trace=bool(int(os.environ.get("KERNEL_TRACE", "0"))),
    )
    _build_and_run.last_results = res

    out = np.concatenate([r["out"].T for r in res.results], axis=0)
    return np.ascontiguousarray(out.astype(np.float32))


def kernel(theta0, context, W1, b1, W2, b2, W3, b3, n_steps):
    return _build_and_run(
        np.asarray(theta0), np.asarray(context), W1, b1, W2, b2, W3, b3, n_steps
    )

